# revision 1
# baseline (speedup 1.0000x reference)
"""Trainium2 Bass kernel for nn_MLPSimDirectNormConstructor (gnn adjacency builder).

adj = [uni_adj(ss) | uni_adj(st); uni_adj(ts) | triu(uni_adj(tt))] for
  spatial_nodes [4,4096,32], temporal_nodes [4,512,32].

Sharding: 8 cores = (batch b = c//2, half h = c%2).  Each core produces
  - 16 interleaved 128-row blocks of the [ss|st] region (rows 128g, g in GL[h])
  - 256 rows of the [ts|tt] region (rows h*256 .. h*256+256)
The interleaved row-block assignment (g%4 in {2h,2h+1}) makes the
upper-triangle-only abs-max scan of the antisymmetric ss block both
load-balanced and SPMD-uniform.

Two collectives: an early AllReduce(max) for the cheap st/ts/tt block maxes
(hidden under the ss max scan) and a late one for ss.  uni_adj scales are
applied via the scalar engine's dynamic per-partition scale/bias operands, so
only the tiny scale vectors depend on the collectives.
"""

import os
import numpy as np
from concourse import bacc, bass_utils, tile, mybir, bass_isa

K_STAGE = int(os.environ.get("K_STAGE", "99"))

B, N, T, D = 4, 4096, 512, 32
NT = N + T
ALPHA = 3.0
EPS = 1e-30
N_CORES = 8
RB = 2048
TB = 256
NBLK = RB // 128
NCH = N // 512
F32 = mybir.dt.float32
F32R = mybir.dt.float32r
TANH = mybir.ActivationFunctionType.Tanh

GL = {h: [g for g in range(N // 128) if (g % 4) // 2 == h] for h in (0, 1)}
JCS = [g // 4 for g in GL[0]]
assert JCS == [g // 4 for g in GL[1]]

# packed small-weights layout (one DMA): [128, WP] f32
#  rows 0:32 -- cols 0:32 w1t, 32:64 w2t, 64:66 wc_st2, 66:68 wc_ts2,
#               68:70 wa_st2, 70:72 wa_ts2
#  all rows  -- col 72 roff; row 0 -- col 73 stb, col 74 tsb
WP = 75


def _build_nc():
    nc = bacc.Bacc(trn_type="TRN2", target_bir_lowering=False, debug=False,
                   num_devices=N_CORES)

    d_in = {}
    for name, shape in [
        ("xs_full", [N, D]), ("xs_rows", [RB, D]),
        ("xt_full", [T, D]), ("xt_rows", [TB, D]),
        ("wpack", [128, WP]),
    ]:
        d_in[name] = nc.dram_tensor(name, shape, F32, kind="ExternalInput")
    out_a = nc.dram_tensor("out_a", [RB, NT], F32, kind="ExternalOutput")
    out_b = nc.dram_tensor("out_b", [TB, NT], F32, kind="ExternalOutput")

    with tile.TileContext(nc) as tc:
        with tc.tile_pool(name="cst", bufs=1) as cst, \
             tc.tile_pool(name="stg", bufs=1) as stg, \
             tc.tile_pool(name="big", bufs=1) as big, \
             tc.tile_pool(name="slabp", bufs=3) as slabp, \
             tc.tile_pool(name="psm", bufs=2, space="PSUM") as psm, \
             tc.tile_pool(name="pss", bufs=2, space="PSUM") as pss, \
             tc.tile_pool(name="pst", bufs=1, space="PSUM") as pst, \
             tc.tile_pool(name="psb", bufs=1, space="PSUM") as psb, \
             tc.tile_pool(name="drm", bufs=1, space="DRAM") as drm:

            # ---------- constants ----------
            onesF = cst.tile([128, 128], F32)
            ident = cst.tile([128, 128], F32)
            nc.vector.memset(onesF[:], 1.0)
            nc.gpsimd.affine_select(ident[:], onesF[:], pattern=[[-1, 128]],
                                    compare_op=mybir.AluOpType.is_equal,
                                    fill=0.0, base=0, channel_multiplier=1)

            # ---------- input DMAs (5 total, all contiguous) ----------
            def ct_load(dram_t, nrows, name):
                k = nrows // 128
                t = stg.tile([128, k * D], F32, tag=name)
                nc.sync.dma_start(out=t[:], in_=dram_t.ap())
                return t

            xs_ct = ct_load(d_in["xs_full"], N, "xs_ct")      # [128, 1024]
            xsr_ct = ct_load(d_in["xs_rows"], RB, "xsr_ct")   # [128, 512]
            xt_ct = ct_load(d_in["xt_full"], T, "xt_ct")      # [128, 128]
            xtr_ct = ct_load(d_in["xt_rows"], TB, "xtr_ct")   # [128, 64]
            wpk = stg.tile([128, WP], F32, tag="wpk")
            nc.sync.dma_start(out=wpk[:], in_=d_in["wpack"].ap())

            w1t_f = wpk[0:D, 0:D]
            w2t_f = wpk[0:D, D:2 * D]
            wc_st2_f = wpk[0:D, 64:66]
            wc_ts2_f = wpk[0:D, 66:68]
            wa_st2_f = wpk[0:D, 68:70]
            wa_ts2_f = wpk[0:D, 70:72]
            roff_sb = wpk[:, 72:73]
            stb_sb = wpk[0:1, 73:74]
            tsb_sb = wpk[0:1, 74:75]

            w1t_r = cst.tile([D, D], F32R)
            w2t_r = cst.tile([D, D], F32R)
            wc_st2_r = cst.tile([D, 2], F32R)
            wc_ts2_r = cst.tile([D, 2], F32R)
            wa_st2_r = cst.tile([D, 2], F32R)
            wa_ts2_r = cst.tile([D, 2], F32R)
            nc.vector.tensor_copy(w1t_r[:], w1t_f)
            nc.vector.tensor_copy(w2t_r[:], w2t_f)
            nc.vector.tensor_copy(wc_st2_r[:], wc_st2_f)
            nc.vector.tensor_copy(wc_ts2_r[:], wc_ts2_f)
            nc.vector.tensor_copy(wa_st2_r[:], wa_st2_f)
            nc.vector.tensor_copy(wa_ts2_r[:], wa_ts2_f)

            # ---------- transposes: contiguous tile -> xT via PE + scatter ---
            # ct[p, r*D+d] = x[K*p + r, d]  (K = nrows/128)
            # transpose of ct[:, c0:c0+w] gives pt[a*D+d, p] = x[K*p + c0/D + a, d]
            # -> rows a*D..a*D+D are xT columns (c0/D + a) with stride K.
            xsT = big.tile([D, N], F32R)
            xsT_rows = big.tile([D, RB], F32R)
            xtT = big.tile([D, T], F32R)
            xtT_rows = big.tile([D, TB], F32R)
            cp_flip = [0]

            def transpose_into(dstT, ct, nrows):
                K = nrows // 128
                total = K * D
                dstv = dstT[:].rearrange("p (n s) -> p n s", s=K)
                for c0 in range(0, total, 128):
                    w = min(128, total - c0)
                    pt = pss.tile([128, 128], F32, tag="sm")
                    nc.tensor.transpose(pt[0:w, :], ct[:, c0:c0 + w], ident[:])
                    for a in range(w // D):
                        colo = c0 // D + a
                        src = pt[D * a:D * a + D, :]
                        dst = dstv[:, :, colo:colo + 1]
                        if cp_flip[0] % 2 == 0:
                            nc.vector.tensor_copy(dst, src)
                        else:
                            nc.scalar.copy(dst, src)
                        cp_flip[0] += 1

            transpose_into(xsT, xs_ct, N)
            transpose_into(xsT_rows, xsr_ct, RB)
            transpose_into(xtT, xt_ct, T)
            transpose_into(xtT_rows, xtr_ct, TB)

            # ---------- n1T/n2T builds (uvL first; per-chunk hi copies) ---
            uvR = big.tile([128, N], F32R)   # [n2T_full ; -n1T_full] x2
            uvL = big.tile([128, RB], F32R)  # [n1T_rows ; n2T_rows] x2
            for jc in range(RB // 512):
                c0 = 512 * jc
                pn = pss.tile([D, 512], F32, tag="sm")
                nc.tensor.matmul(pn[:], w1t_r[:], xsT_rows[:, c0:c0 + 512],
                                 start=True, stop=True)
                nc.scalar.activation(uvL[0:D, c0:c0 + 512], pn[:], TANH,
                                     bias=0.0, scale=ALPHA)
                pn2 = pss.tile([D, 512], F32, tag="sm")
                nc.tensor.matmul(pn2[:], w2t_r[:], xsT_rows[:, c0:c0 + 512],
                                 start=True, stop=True)
                nc.scalar.activation(uvL[D:2 * D, c0:c0 + 512], pn2[:], TANH,
                                     bias=0.0, scale=ALPHA)
                nc.sync.dma_start(out=uvL[64:128, c0:c0 + 512],
                                  in_=uvL[0:64, c0:c0 + 512])
            for jc in range(NCH):
                c0 = 512 * jc
                pn = pss.tile([D, 512], F32, tag="sm")
                nc.tensor.matmul(pn[:], w2t_r[:], xsT[:, c0:c0 + 512],
                                 start=True, stop=True)
                nc.scalar.activation(uvR[0:D, c0:c0 + 512], pn[:], TANH,
                                     bias=0.0, scale=ALPHA)
                pn2 = pss.tile([D, 512], F32, tag="sm")
                nc.tensor.matmul(pn2[:], w1t_r[:], xsT[:, c0:c0 + 512],
                                 start=True, stop=True)
                nc.scalar.activation(uvR[D:2 * D, c0:c0 + 512], pn2[:], TANH,
                                     bias=0.0, scale=-ALPHA)
                nc.sync.dma_start(out=uvR[64:128, c0:c0 + 512],
                                  in_=uvR[0:64, c0:c0 + 512])

            # ---------- c vectors ----------
            c_st = big.tile([1, T], F32)
            c_ts = big.tile([1, N], F32)
            pg = pss.tile([2, 512], F32, tag="sm")
            nc.tensor.matmul(pg[:], wc_st2_r[:], xtT[:], start=True, stop=True)
            nc.vector.tensor_copy(c_st[0:1, :], pg[0:1, :])
            for jc in range(NCH):
                c0 = 512 * jc
                pg2 = pss.tile([2, 512], F32, tag="sm")
                nc.tensor.matmul(pg2[:], wc_ts2_r[:], xsT[:, c0:c0 + 512],
                                 start=True, stop=True)
                nc.vector.tensor_copy(c_ts[0:1, c0:c0 + 512], pg2[0:1, :])

            # ---------- a vectors: per-slab gemv, partition-major ----------
            # a_st_pm[p, i] = a_st[128*i + p]
            a_st_pm = big.tile([128, NBLK], F32)
            a_ts_pm = big.tile([128, 2], F32)
            for i in range(NBLK):
                pa = pss.tile([128, 2], F32, tag="sm")
                nc.tensor.matmul(pa[:], xsT_rows[:, 128 * i:128 * i + 128],
                                 wa_st2_r[:], start=True, stop=True)
                nc.vector.tensor_copy(a_st_pm[:, i:i + 1], pa[:, 0:1])
            for m in range(2):
                pa = pss.tile([128, 2], F32, tag="sm")
                nc.tensor.matmul(pa[:], xtT_rows[:, 128 * m:128 * m + 128],
                                 wa_ts2_r[:], start=True, stop=True)
                nc.vector.tensor_copy(a_ts_pm[:, m:m + 1], pa[:, 0:1])

            # (c+bias) moving rows for the K=1 st/ts matmuls (pre-collective)
            rhs_st0 = big.tile([1, T], F32R)
            rhs_ts0 = big.tile([1, N], F32R)
            nc.vector.tensor_scalar(rhs_st0[:], c_st[0:1, :], stb_sb, None,
                                    mybir.AluOpType.add)
            nc.vector.tensor_scalar(rhs_ts0[:], c_ts[0:1, :], tsb_sb, None,
                                    mybir.AluOpType.add)
            ones_lhsT = big.tile([1, 128], F32R)
            nc.vector.tensor_scalar(ones_lhsT[:], xsT[0:1, 0:128], 0.0, 1.0,
                                    mybir.AluOpType.mult, mybir.AluOpType.add)
            st_ps = pst.tile([128, 512], F32)
            nc.tensor.matmul(st_ps[:], ones_lhsT[:], rhs_st0[:],
                             start=True, stop=True)

            # ---------- tt triu masks ----------
            msks = []
            for m in range(2):
                itF = stg.tile([128, 512], F32, tag="itF")
                nc.gpsimd.iota(itF[:], pattern=[[1, 512]], base=-128 * m,
                               channel_multiplier=-1,
                               allow_small_or_imprecise_dtypes=True)
                msk = big.tile([128, 512], F32, tag=f"msk{m}")
                nc.vector.tensor_scalar(msk[:], itF[:], roff_sb, None,
                                        mybir.AluOpType.is_ge)
                msks.append(msk)

            # ---------- early partials (st/ts/tt) + collective A --------
            ttmaxb = big.tile([128, 2], F32)
            for m in range(2):
                pm_ = psm.tile([128, 1024], F32, tag="mm")
                nc.tensor.matmul(pm_[:, 0:512],
                                 xtT_rows[:, 128 * m:128 * m + 128],
                                 xtT[:], start=True, stop=True)
                nc.vector.tensor_reduce(ttmaxb[:, m:m + 1], pm_[:, 0:512],
                                        axis=mybir.AxisListType.X,
                                        op=mybir.AluOpType.max)
            partA = big.tile([128, 3], F32)
            nc.vector.memset(partA[:], 0.0)
            maxa_st = big.tile([128, 1], F32)
            maxa_ts = big.tile([128, 1], F32)
            nc.vector.tensor_reduce(maxa_st[:], a_st_pm[:],
                                    axis=mybir.AxisListType.X,
                                    op=mybir.AluOpType.max)
            nc.vector.tensor_reduce(maxa_ts[:], a_ts_pm[:],
                                    axis=mybir.AxisListType.X,
                                    op=mybir.AluOpType.max)
            maxa_st_r = big.tile([128, 1], F32)
            maxa_ts_r = big.tile([128, 1], F32)
            nc.gpsimd.partition_all_reduce(maxa_st_r[:], maxa_st[:],
                                           channels=128,
                                           reduce_op=bass_isa.ReduceOp.max)
            nc.gpsimd.partition_all_reduce(maxa_ts_r[:], maxa_ts[:],
                                           channels=128,
                                           reduce_op=bass_isa.ReduceOp.max)
            maxc_st = big.tile([1, 1], F32)
            maxc_ts = big.tile([1, 1], F32)
            nc.vector.tensor_reduce(maxc_st[:], c_st[0:1, :],
                                    axis=mybir.AxisListType.X,
                                    op=mybir.AluOpType.max)
            nc.vector.tensor_reduce(maxc_ts[:], c_ts[0:1, :],
                                    axis=mybir.AxisListType.X,
                                    op=mybir.AluOpType.max)
            tmp_st = big.tile([1, 1], F32)
            tmp_ts = big.tile([1, 1], F32)
            nc.vector.tensor_tensor(tmp_st[:], maxa_st_r[0:1, 0:1],
                                    maxc_st[:], mybir.AluOpType.add)
            nc.vector.tensor_tensor(partA[0:1, 0:1], tmp_st[:], stb_sb,
                                    mybir.AluOpType.add)
            nc.vector.tensor_tensor(tmp_ts[:], maxa_ts_r[0:1, 0:1],
                                    maxc_ts[:], mybir.AluOpType.add)
            nc.vector.tensor_tensor(partA[0:1, 1:2], tmp_ts[:], tsb_sb,
                                    mybir.AluOpType.add)
            nc.vector.tensor_reduce(partA[:, 2:3], ttmaxb[:],
                                    axis=mybir.AxisListType.X,
                                    op=mybir.AluOpType.max)
            nc.vector.tensor_scalar_max(partA[:], partA[:], 0.0)
            partA_r = big.tile([128, 3], F32)
            nc.gpsimd.partition_all_reduce(partA_r[:], partA[:],
                                           channels=128,
                                           reduce_op=bass_isa.ReduceOp.max)
            binA = drm.tile([128, 3], F32)
            boutA = drm.tile([128, 3], F32)
            nc.sync.dma_start(out=binA[:], in_=partA_r[:])
            nc.gpsimd.collective_compute(
                "AllReduce", mybir.AluOpType.max,
                replica_groups=[list(range(N_CORES))],
                ins=[binA.opt()], outs=[boutA.opt()])
            # ---------- pass 1: ss abs-max + collective B ----------
            tiles1 = [(i, jc) for i in range(NBLK)
                      for jc in range(JCS[i], NCH)]
            n_pair = len(tiles1) // 2
            maxbuf = big.tile([128, n_pair], F32)
            for t in range(n_pair):
                iA, jA = tiles1[2 * t]
                iB, jB = tiles1[2 * t + 1]
                pm_ = psm.tile([128, 1024], F32, tag="mm")
                nc.tensor.matmul(pm_[:, 0:512],
                                 uvL[0:64, 128 * iA:128 * iA + 128],
                                 uvR[0:64, 512 * jA:512 * jA + 512],
                                 start=True, stop=True,
                                 tile_position=(0, 0))
                nc.tensor.matmul(pm_[:, 512:1024],
                                 uvL[64:128, 128 * iB:128 * iB + 128],
                                 uvR[64:128, 512 * jB:512 * jB + 512],
                                 start=True, stop=True,
                                 tile_position=(64, 0))
                nc.vector.tensor_reduce(maxbuf[:, t:t + 1], pm_[:],
                                        axis=mybir.AxisListType.X,
                                        op=mybir.AluOpType.max,
                                        apply_absolute_value=True)
            gmaxA = big.tile([128, 3], F32)
            nc.sync.dma_start(out=gmaxA[:], in_=boutA[:])
            t3 = big.tile([128, 3], F32)
            nc.vector.tensor_scalar_add(t3[:], gmaxA[:], EPS)
            scales3 = big.tile([128, 3], F32)
            nc.vector.reciprocal(scales3[:], t3[:])
            sa_st = big.tile([128, NBLK], F32)
            sa_ts = big.tile([128, 2], F32)
            nc.vector.tensor_scalar_mul(sa_st[:], a_st_pm[:],
                                        scales3[:, 0:1])
            nc.vector.tensor_scalar_mul(sa_ts[:], a_ts_pm[:],
                                        scales3[:, 1:2])

            part1 = big.tile([128, 1], F32)
            nc.vector.tensor_reduce(part1[:], maxbuf[:],
                                    axis=mybir.AxisListType.X,
                                    op=mybir.AluOpType.max)
            part1_r = big.tile([128, 1], F32)
            nc.gpsimd.partition_all_reduce(part1_r[:], part1[:],
                                           channels=128,
                                           reduce_op=bass_isa.ReduceOp.max)
            binB = drm.tile([128, 1], F32)
            boutB = drm.tile([128, 1], F32)
            nc.sync.dma_start(out=binB[:], in_=part1_r[:])
            nc.gpsimd.collective_compute(
                "AllReduce", mybir.AluOpType.max,
                replica_groups=[list(range(N_CORES))],
                ins=[binB.opt()], outs=[boutB.opt()])
            gmaxB = big.tile([128, 1], F32)
            nc.sync.dma_start(out=gmaxB[:], in_=boutB[:])
            t1 = big.tile([128, 1], F32)
            nc.vector.tensor_scalar(t1[:], gmaxB[:], ALPHA, EPS,
                                    mybir.AluOpType.mult,
                                    mybir.AluOpType.add)
            rec1 = big.tile([128, 1], F32)
            nc.vector.reciprocal(rec1[:], t1[:])
            s_ss = big.tile([128, 1], F32)
            nc.vector.tensor_scalar_mul(s_ss[:], rec1[:], ALPHA)

            # ---------- pass 2B: [ts | tt] (gated on collective A only) --
            for m in range(2):
                slab = slabp.tile([128, NT], F32, tag="slab")
                for jc in range(NCH):
                    c0 = 512 * jc
                    pm_ = psb.tile([128, 512], F32, tag="mmb")
                    nc.tensor.matmul(pm_[:], ones_lhsT[:],
                                     rhs_ts0[0:1, c0:c0 + 512],
                                     start=True, stop=True)
                    nc.scalar.activation(slab[:, c0:c0 + 512], pm_[:],
                                         TANH, bias=sa_ts[:, m:m + 1],
                                         scale=scales3[:, 1:2])
                pm_ = psb.tile([128, 512], F32, tag="mmb")
                nc.tensor.matmul(pm_[:],
                                 xtT_rows[:, 128 * m:128 * m + 128],
                                 xtT[:], start=True, stop=True)
                nc.scalar.activation(slab[:, N:NT], pm_[:], TANH,
                                     bias=0.0, scale=scales3[:, 2:3])
                nc.vector.tensor_scalar_max(slab[:], slab[:], 0.0)
                nc.vector.tensor_tensor(slab[:, N:NT], slab[:, N:NT],
                                        msks[m][:], mybir.AluOpType.mult)
                nc.sync.dma_start(out=out_b.ap()[128 * m:128 * m + 128, :],
                                  in_=slab[:])

            # ---------- pass 2A: [ss | st] ----------
            for i in range(NBLK):
                slab = slabp.tile([128, NT], F32, tag="slab")
                for jc in range(0, NCH, 2):
                    c0 = 512 * jc
                    pm_ = psm.tile([128, 1024], F32, tag="mm")
                    nc.tensor.matmul(pm_[:, 0:512],
                                     uvL[0:64, 128 * i:128 * i + 128],
                                     uvR[0:64, c0:c0 + 512],
                                     start=True, stop=True,
                                     tile_position=(0, 0))
                    nc.tensor.matmul(pm_[:, 512:1024],
                                     uvL[64:128, 128 * i:128 * i + 128],
                                     uvR[64:128, c0 + 512:c0 + 1024],
                                     start=True, stop=True,
                                     tile_position=(64, 0))
                    nc.scalar.activation(slab[:, c0:c0 + 1024], pm_[:],
                                         TANH, bias=0.0, scale=s_ss[:, 0:1])
                nc.scalar.activation(slab[:, N:NT], st_ps[:], TANH,
                                     bias=sa_st[:, i:i + 1],
                                     scale=scales3[:, 0:1])
                nc.vector.tensor_scalar_max(slab[:], slab[:], 0.0)
                nc.sync.dma_start(out=out_a.ap()[128 * i:128 * i + 128, :],
                                  in_=slab[:])

    nc.finalize()
    return nc


def _in_maps(spatial_nodes, temporal_nodes, ss1_w, ss2_w, st_w, st_b, ts_w, ts_b):
    f = np.float32
    maps = []
    wpack = np.zeros((128, WP), dtype=f)
    wpack[0:D, 0:D] = ss1_w.T
    wpack[0:D, D:2 * D] = ss2_w.T
    wpack[0:D, 64:66] = np.stack([st_w[0, D:], st_w[0, D:]], 1)
    wpack[0:D, 66:68] = np.stack([ts_w[0, D:], ts_w[0, D:]], 1)
    wpack[0:D, 68:70] = np.stack([st_w[0, :D], st_w[0, :D]], 1)
    wpack[0:D, 70:72] = np.stack([ts_w[0, :D], ts_w[0, :D]], 1)
    wpack[0, 73] = np.float32(np.asarray(st_b).reshape(-1)[0])
    wpack[0, 74] = np.float32(np.asarray(ts_b).reshape(-1)[0])
    for c in range(N_CORES):
        b, h = c // 2, c % 2
        wp = wpack.copy()
        wp[:, 72] = TB * h
        xs_b = np.asarray(spatial_nodes[b], dtype=f)
        xt_b = np.asarray(temporal_nodes[b], dtype=f)
        xs_rows = np.ascontiguousarray(
            np.concatenate([xs_b[128 * g:128 * g + 128] for g in GL[h]], 0))
        maps.append({
            "xs_full": np.ascontiguousarray(xs_b),
            "xs_rows": xs_rows,
            "xt_full": np.ascontiguousarray(xt_b),
            "xt_rows": np.ascontiguousarray(xt_b[TB * h:TB * h + TB]),
            "wpack": wp,
        })
    return maps


def run_kernel(inputs, trace=False, **spmd_kwargs):
    nc = _build_nc()
    maps = _in_maps(**inputs)
    res = bass_utils.run_bass_kernel_spmd(
        nc, maps, core_ids=list(range(N_CORES)), trace=trace, **spmd_kwargs)
    adj = np.empty((B, NT, NT), dtype=np.float32)
    for c in range(N_CORES):
        b, h = c // 2, c % 2
        oa = res.results[c]["out_a"]
        ob = res.results[c]["out_b"]
        for li, g in enumerate(GL[h]):
            adj[b, 128 * g:128 * g + 128, :] = oa[128 * li:128 * li + 128]
        adj[b, N + TB * h:N + TB * h + TB, :] = ob
    return adj, res


def kernel(**inputs):
    adj, _ = run_kernel(inputs, trace=False)
    return adj



# revision 4
# speedup vs baseline: 1.3172x; 1.3172x over previous
"""Trainium2 Bass kernel for nn_MLPSimDirectNormConstructor (gnn adjacency builder).

adj = [uni_adj(ss) | uni_adj(st); uni_adj(ts) | triu(uni_adj(tt))] for
  spatial_nodes [4,4096,32], temporal_nodes [4,512,32].

Sharding: 8 cores = (batch b = c//2, half h = c%2).  Each core produces
  - 16 interleaved 128-row blocks of the [ss|st] region (rows 128g, g in GL[h])
  - 256 rows of the [ts|tt] region (rows h*256 .. h*256+256)
The interleaved row-block assignment (g%4 in {2h,2h+1}) makes the
upper-triangle-only abs-max scan of the antisymmetric ss block both
load-balanced and SPMD-uniform.

v2 changes vs the first working version:
  - all x transposes are done on the host (pure layout prep) - no PE
    transpose + copy phase on device
  - outputs are fp16 (values are in [0,1] post-relu; fp16 quantization is
    ~2.4e-4 abs, well under the 2e-2 gate); host converts back to f32.
    Halves the dominant HBM write traffic.
  - uvL/uvR (the tanh-projected factors of the antisymmetric ss block) are
    stored bf16
  - PSUM is used as 2 x [128,2048] tiles; one activation instruction per
    2048 columns (amortizes the ~222-cycle ACT SBUF access latency)
  - two max-collectives as before (st/ts/tt early, ss late), but the
    collective-B latency is filled with collective-A-gated work: the whole
    [ts|tt] pass plus the st columns of the first 8 [ss|st] slabs.
"""

import numpy as np
from concourse import bacc, bass_utils, tile, mybir, bass_isa

B, N, T, D = 4, 4096, 512, 32
NT = N + T
ALPHA = 3.0
EPS = 1e-30
N_CORES = 8
RB = 2048          # ss|st rows per core
TB = 256           # ts|tt rows per core
NBLK = RB // 128   # 16 row blocks
NCH = N // 512     # 8 column chunks of 512
F32 = mybir.dt.float32
F32R = mybir.dt.float32r
BF16 = mybir.dt.bfloat16
F16 = mybir.dt.float16
TANH = mybir.ActivationFunctionType.Tanh

GL = {h: [g for g in range(N // 128) if (g % 4) // 2 == h] for h in (0, 1)}
JCS = [g // 4 for g in GL[0]]
assert JCS == [g // 4 for g in GL[1]]

# packed small-weights layout (one DMA): [128, WP] f32
#  rows 0:32 -- cols 0:64  w12T  ([ss1_w.T | ss2_w.T], lhsT for uvL)
#  rows 0:32 -- cols 64:128 w21T ([ss2_w.T | ss1_w.T], lhsT for uvR)
#  rows 0:32 -- cols 128:130 wc_st2, 130:132 wc_ts2, 132:134 wa_st2,
#               134:136 wa_ts2
#  all rows  -- col 136 roff; col 137 svec (+a/-a); row 0 -- col 138 stb,
#               col 139 tsb
WP = 140


def _build_nc():
    nc = bacc.Bacc(trn_type="TRN2", target_bir_lowering=False, debug=False,
                   num_devices=N_CORES)

    d_in = {}
    for name, shape in [
        ("xsT", [D, N]), ("xsrT", [D, RB]),
        ("xtT", [D, T]), ("xtrT", [D, TB]),
    ]:
        d_in[name] = nc.dram_tensor(name, shape, F32R, kind="ExternalInput")
    d_in["wpack"] = nc.dram_tensor("wpack", [128, WP], F32,
                                   kind="ExternalInput")
    out_a = nc.dram_tensor("out_a", [RB, NT], F16, kind="ExternalOutput")
    out_b = nc.dram_tensor("out_b", [TB, NT], F16, kind="ExternalOutput")

    with tile.TileContext(nc) as tc:
        with tc.tile_pool(name="stg", bufs=1) as stg, \
             tc.tile_pool(name="big", bufs=1) as big, \
             tc.tile_pool(name="slabp", bufs=8) as slabp, \
             tc.tile_pool(name="slabq", bufs=2) as slabq, \
             tc.tile_pool(name="psm", bufs=2, space="PSUM") as psm, \
             tc.tile_pool(name="drm", bufs=1, space="DRAM") as drm:

            # ---------- input DMAs (5 total, all contiguous) ----------
            xsT = stg.tile([D, N], F32R, tag="xsT")
            xsrT = stg.tile([D, RB], F32R, tag="xsrT")
            xtT = stg.tile([D, T], F32R, tag="xtT")
            xtrT = stg.tile([D, TB], F32R, tag="xtrT")
            wpk = stg.tile([128, WP], F32, tag="wpk")
            for t, d in [(xsT, "xsT"), (xsrT, "xsrT"), (xtT, "xtT"),
                         (xtrT, "xtrT"), (wpk, "wpack")]:
                nc.sync.dma_start(out=t[:], in_=d_in[d].ap())

            w12_f = wpk[0:D, 0:64]
            w21_f = wpk[0:D, 64:128]
            wc_st2_f = wpk[0:D, 128:130]
            wc_ts2_f = wpk[0:D, 130:132]
            wa_st2_f = wpk[0:D, 132:134]
            wa_ts2_f = wpk[0:D, 134:136]
            roff_sb = wpk[:, 136:137]
            svec = wpk[:, 137:138]      # rows 0:32 = +ALPHA, 32:64 = -ALPHA
            stb_sb = wpk[0:1, 138:139]
            tsb_sb = wpk[0:1, 139:140]

            w12_r = big.tile([D, 64], F32R, tag="w12r")
            w21_r = big.tile([D, 64], F32R, tag="w21r")
            wc_st2_r = big.tile([D, 2], F32R, tag="wcst")
            wc_ts2_r = big.tile([D, 2], F32R, tag="wcts")
            wa_st2_r = big.tile([D, 2], F32R, tag="wast")
            wa_ts2_r = big.tile([D, 2], F32R, tag="wats")
            nc.vector.tensor_copy(w12_r[:], w12_f)
            nc.vector.tensor_copy(w21_r[:], w21_f)
            nc.vector.tensor_copy(wc_st2_r[:], wc_st2_f)
            nc.vector.tensor_copy(wc_ts2_r[:], wc_ts2_f)
            nc.vector.tensor_copy(wa_st2_r[:], wa_st2_f)
            nc.vector.tensor_copy(wa_ts2_r[:], wa_ts2_f)

            # ---------- uv builds ----------
            # uvL = [n1T_rows ; n2T_rows] x2 (bf16), uvR = [n2T ; -n1T] x2
            uvL = big.tile([128, RB], BF16, tag="uvL")
            uvR = big.tile([128, N], BF16, tag="uvR")
            pL = psm.tile([128, 2048], F32, tag="mm")
            for j in range(4):
                nc.tensor.matmul(pL[0:64, 512 * j:512 * j + 512], w12_r[:],
                                 xsrT[:, 512 * j:512 * j + 512],
                                 start=True, stop=True)
            nc.scalar.activation(uvL[0:64, :], pL[0:64, :], TANH,
                                 bias=0.0, scale=ALPHA)
            for half in range(2):
                pR = psm.tile([128, 2048], F32, tag="mm")
                for j in range(4):
                    c0 = 2048 * half + 512 * j
                    nc.tensor.matmul(pR[0:64, 512 * j:512 * j + 512],
                                     w21_r[:], xsT[:, c0:c0 + 512],
                                     start=True, stop=True)
                nc.scalar.activation(uvR[0:64, 2048 * half:2048 * half + 2048],
                                     pR[0:64, :], TANH, bias=0.0,
                                     scale=svec[0:64, 0:1])
            nc.sync.dma_start(out=uvL[64:128, :], in_=uvL[0:64, :])
            nc.sync.dma_start(out=uvR[64:128, :], in_=uvR[0:64, :])

            # ---------- c vectors, a vectors, tt partials (cheap, early) ---
            c_st = big.tile([1, T], F32, tag="cst")
            c_ts = big.tile([1, N], F32, tag="cts")
            pg = psm.tile([128, 2048], F32, tag="mm")
            nc.tensor.matmul(pg[0:2, 0:512], wc_st2_r[:], xtT[:],
                             start=True, stop=True)
            nc.vector.tensor_copy(c_st[0:1, :], pg[0:1, 0:512])
            for half in range(2):
                pg2 = psm.tile([128, 2048], F32, tag="mm")
                for jc in range(4):
                    nc.tensor.matmul(pg2[0:2, 512 * jc:512 * jc + 512],
                                     wc_ts2_r[:],
                                     xsT[:, 2048 * half + 512 * jc:
                                         2048 * half + 512 * jc + 512],
                                     start=True, stop=True)
                nc.vector.tensor_copy(c_ts[0:1, 2048 * half:2048 * half + 2048],
                                      pg2[0:1, 0:2048])

            # a vectors partition-major: a_st_pm[p, i] = a_st[128*i + p]
            a_st_pm = big.tile([128, NBLK], F32, tag="astpm")
            a_ts_pm = big.tile([128, 2], F32, tag="atspm")
            ttmaxb = big.tile([128, 2], F32, tag="ttmaxb")
            pa = psm.tile([128, 2048], F32, tag="mm")
            for i in range(NBLK):
                nc.tensor.matmul(pa[:, 2 * i:2 * i + 2],
                                 xsrT[:, 128 * i:128 * i + 128],
                                 wa_st2_r[:], start=True, stop=True)
            for m in range(2):
                nc.tensor.matmul(pa[:, 32 + 2 * m:34 + 2 * m],
                                 xtrT[:, 128 * m:128 * m + 128],
                                 wa_ts2_r[:], start=True, stop=True)
            nc.vector.tensor_copy(
                a_st_pm[:], pa[:, 0:32].rearrange("p (n s) -> p n s", s=2)[:, :, 0:1])
            nc.vector.tensor_copy(
                a_ts_pm[:], pa[:, 32:36].rearrange("p (n s) -> p n s", s=2)[:, :, 0:1])
            ptt = psm.tile([128, 2048], F32, tag="mm")
            for m in range(2):
                nc.tensor.matmul(ptt[:, 512 * m:512 * m + 512],
                                 xtrT[:, 128 * m:128 * m + 128],
                                 xtT[:], start=True, stop=True)
                nc.vector.tensor_reduce(ttmaxb[:, m:m + 1],
                                        ptt[:, 512 * m:512 * m + 512],
                                        axis=mybir.AxisListType.X,
                                        op=mybir.AluOpType.max)

            # ---------- partial maxes + collective A (st/ts/tt) ----------
            partA = big.tile([128, 3], F32, tag="partA")
            nc.vector.memset(partA[:], 0.0)
            maxa_st = big.tile([128, 1], F32, tag="maxast")
            maxa_ts = big.tile([128, 1], F32, tag="maxats")
            nc.vector.tensor_reduce(maxa_st[:], a_st_pm[:],
                                    axis=mybir.AxisListType.X,
                                    op=mybir.AluOpType.max)
            nc.vector.tensor_reduce(maxa_ts[:], a_ts_pm[:],
                                    axis=mybir.AxisListType.X,
                                    op=mybir.AluOpType.max)
            maxa_st_r = big.tile([128, 1], F32, tag="maxastr")
            maxa_ts_r = big.tile([128, 1], F32, tag="maxatsr")
            nc.gpsimd.partition_all_reduce(maxa_st_r[:], maxa_st[:],
                                           channels=128,
                                           reduce_op=bass_isa.ReduceOp.max)
            nc.gpsimd.partition_all_reduce(maxa_ts_r[:], maxa_ts[:],
                                           channels=128,
                                           reduce_op=bass_isa.ReduceOp.max)
            maxc_st = big.tile([1, 1], F32, tag="maxcst")
            maxc_ts = big.tile([1, 1], F32, tag="maxcts")
            nc.vector.tensor_reduce(maxc_st[:], c_st[0:1, :],
                                    axis=mybir.AxisListType.X,
                                    op=mybir.AluOpType.max)
            nc.vector.tensor_reduce(maxc_ts[:], c_ts[0:1, :],
                                    axis=mybir.AxisListType.X,
                                    op=mybir.AluOpType.max)
            tmp_st = big.tile([1, 1], F32, tag="tmpst")
            tmp_ts = big.tile([1, 1], F32, tag="tmpts")
            nc.vector.tensor_tensor(tmp_st[:], maxa_st_r[0:1, 0:1],
                                    maxc_st[:], mybir.AluOpType.add)
            nc.vector.tensor_tensor(partA[0:1, 0:1], tmp_st[:], stb_sb,
                                    mybir.AluOpType.add)
            nc.vector.tensor_tensor(tmp_ts[:], maxa_ts_r[0:1, 0:1],
                                    maxc_ts[:], mybir.AluOpType.add)
            nc.vector.tensor_tensor(partA[0:1, 1:2], tmp_ts[:], tsb_sb,
                                    mybir.AluOpType.add)
            nc.vector.tensor_reduce(partA[:, 2:3], ttmaxb[:],
                                    axis=mybir.AxisListType.X,
                                    op=mybir.AluOpType.max)
            nc.vector.tensor_scalar_max(partA[:], partA[:], 0.0)
            partA_r = big.tile([128, 3], F32, tag="partAr")
            nc.gpsimd.partition_all_reduce(partA_r[:], partA[:],
                                           channels=128,
                                           reduce_op=bass_isa.ReduceOp.max)
            binA = drm.tile([128, 3], F32, tag="binA")
            boutA = drm.tile([128, 3], F32, tag="boutA")
            nc.sync.dma_start(out=binA[:], in_=partA_r[:])
            nc.gpsimd.collective_compute(
                "AllReduce", mybir.AluOpType.max,
                replica_groups=[list(range(N_CORES))],
                ins=[binA.opt()], outs=[boutA.opt()])

            # ---------- ss abs-max scan + collective B ----------
            tiles1 = [(i, jc) for i in range(NBLK)
                      for jc in range(JCS[i], NCH)]
            assert len(tiles1) % 4 == 0
            n_quad = len(tiles1) // 4
            maxbuf = big.tile([128, n_quad], F32, tag="maxbuf")
            for t in range(n_quad):
                pm_ = psm.tile([128, 2048], F32, tag="mm")
                for s in range(4):
                    i, jc = tiles1[4 * t + s]
                    q = 64 * (s % 2)
                    nc.tensor.matmul(pm_[:, 512 * s:512 * s + 512],
                                     uvL[q:q + 64, 128 * i:128 * i + 128],
                                     uvR[q:q + 64, 512 * jc:512 * jc + 512],
                                     start=True, stop=True,
                                     tile_position=(q, 0))
                nc.vector.tensor_reduce(maxbuf[:, t:t + 1], pm_[:],
                                        axis=mybir.AxisListType.X,
                                        op=mybir.AluOpType.max,
                                        apply_absolute_value=True)
            part1 = big.tile([128, 1], F32, tag="part1")
            nc.vector.tensor_reduce(part1[:], maxbuf[:],
                                    axis=mybir.AxisListType.X,
                                    op=mybir.AluOpType.max)
            part1_r = big.tile([128, 1], F32, tag="part1r")
            nc.gpsimd.partition_all_reduce(part1_r[:], part1[:],
                                           channels=128,
                                           reduce_op=bass_isa.ReduceOp.max)
            binB = drm.tile([128, 1], F32, tag="binB")
            boutB = drm.tile([128, 1], F32, tag="boutB")
            nc.sync.dma_start(out=binB[:], in_=part1_r[:])
            nc.gpsimd.collective_compute(
                "AllReduce", mybir.AluOpType.max,
                replica_groups=[list(range(N_CORES))],
                ins=[binB.opt()], outs=[boutB.opt()])

            # ---------- consume collective A ----------
            gmaxA = big.tile([128, 3], F32, tag="gmaxA")
            nc.sync.dma_start(out=gmaxA[:], in_=boutA[:])
            t3 = big.tile([128, 3], F32, tag="t3")
            nc.vector.tensor_scalar_add(t3[:], gmaxA[:], EPS)
            scales3 = big.tile([128, 3], F32, tag="scales3")
            nc.vector.reciprocal(scales3[:], t3[:])
            sa_st = big.tile([128, NBLK], F32, tag="sast")
            sa_ts = big.tile([128, 2], F32, tag="sats")
            nc.vector.tensor_scalar_mul(sa_st[:], a_st_pm[:],
                                        scales3[:, 0:1])
            nc.vector.tensor_scalar_mul(sa_ts[:], a_ts_pm[:],
                                        scales3[:, 1:2])

            # (c+bias) moving rows for the K=1 st/ts matmuls
            rhs_st0 = big.tile([1, T], F32R, tag="rhsst")
            rhs_ts0 = big.tile([1, N], F32R, tag="rhsts")
            nc.vector.tensor_scalar(rhs_st0[:], c_st[0:1, :], stb_sb, None,
                                    mybir.AluOpType.add)
            nc.vector.tensor_scalar(rhs_ts0[:], c_ts[0:1, :], tsb_sb, None,
                                    mybir.AluOpType.add)
            ones_lhsT = big.tile([1, 128], F32R, tag="ones")
            nc.vector.tensor_scalar(ones_lhsT[:], xsT[0:1, 0:128], 0.0, 1.0,
                                    mybir.AluOpType.mult, mybir.AluOpType.add)

            # tt triu masks (fp16)
            msks = []
            for m in range(2):
                itF = big.tile([128, 512], F32, tag=f"itF{m}")
                nc.gpsimd.iota(itF[:], pattern=[[1, 512]], base=-128 * m,
                               channel_multiplier=-1,
                               allow_small_or_imprecise_dtypes=True)
                msk = big.tile([128, 512], F16, tag=f"msk{m}")
                nc.vector.tensor_scalar(msk[:], itF[:], roff_sb, None,
                                        mybir.AluOpType.is_ge)
                msks.append(msk)

            # ---------- pass 2B: [ts | tt] (gated on collective A only) --
            for m in range(2):
                slab = slabq.tile([128, NT], F16, tag="slabB")
                for half in range(2):
                    pm_ = psm.tile([128, 2048], F32, tag="mm")
                    for j in range(4):
                        c0 = 2048 * half + 512 * j
                        nc.tensor.matmul(pm_[:, 512 * j:512 * j + 512],
                                         ones_lhsT[:],
                                         rhs_ts0[0:1, c0:c0 + 512],
                                         start=True, stop=True)
                    nc.scalar.activation(
                        slab[:, 2048 * half:2048 * half + 2048], pm_[:],
                        TANH, bias=sa_ts[:, m:m + 1], scale=scales3[:, 1:2])
                pm_ = psm.tile([128, 2048], F32, tag="mm")
                nc.tensor.matmul(pm_[:, 0:512],
                                 xtrT[:, 128 * m:128 * m + 128],
                                 xtT[:], start=True, stop=True)
                nc.scalar.activation(slab[:, N:NT], pm_[:, 0:512], TANH,
                                     bias=0.0, scale=scales3[:, 2:3])
                nc.vector.tensor_scalar_max(slab[:], slab[:], 0.0)
                nc.vector.tensor_tensor(slab[:, N:NT], slab[:, N:NT],
                                        msks[m][:], mybir.AluOpType.mult)
                nc.sync.dma_start(out=out_b.ap()[128 * m:128 * m + 128, :],
                                  in_=slab[:])

            # ---------- st prefill for first 8 slabs (collective-A gated) --
            # Fills the [st] columns of slabs 0..7 while collective B is in
            # flight; their [ss] columns are written post-collective-B.
            slabs = [slabp.tile([128, NT], F16, tag="slab", name=f"slab{k}")
                     for k in range(8)]
            PRE = 8

            def st_fill(slab, i):
                pm_ = psm.tile([128, 2048], F32, tag="mm")
                nc.tensor.matmul(pm_[:, 0:512], ones_lhsT[:], rhs_st0[:],
                                 start=True, stop=True)
                nc.scalar.activation(slab[:, N:NT], pm_[:, 0:512], TANH,
                                     bias=sa_st[:, i:i + 1],
                                     scale=scales3[:, 0:1])

            for i in range(PRE):
                st_fill(slabs[i], i)

            # ---------- consume collective B ----------
            gmaxB = big.tile([128, 1], F32, tag="gmaxB")
            nc.sync.dma_start(out=gmaxB[:], in_=boutB[:])
            t1 = big.tile([128, 1], F32, tag="t1")
            nc.vector.tensor_scalar(t1[:], gmaxB[:], ALPHA, EPS,
                                    mybir.AluOpType.mult,
                                    mybir.AluOpType.add)
            rec1 = big.tile([128, 1], F32, tag="rec1")
            nc.vector.reciprocal(rec1[:], t1[:])
            s_ss = big.tile([128, 1], F32, tag="sss")
            nc.vector.tensor_scalar_mul(s_ss[:], rec1[:], ALPHA)

            # ---------- main pass: [ss | st] slabs ----------
            for i in range(NBLK):
                if i < PRE:
                    slab = slabs[i]
                else:
                    slab = slabp.tile([128, NT], F16, tag="slab")
                    st_fill(slab, i)
                for half in range(2):
                    pm_ = psm.tile([128, 2048], F32, tag="mm")
                    for s in range(4):
                        jc = 4 * half + s
                        q = 64 * (s % 2)
                        nc.tensor.matmul(pm_[:, 512 * s:512 * s + 512],
                                         uvL[q:q + 64, 128 * i:128 * i + 128],
                                         uvR[q:q + 64, 512 * jc:512 * jc + 512],
                                         start=True, stop=True,
                                         tile_position=(q, 0))
                    nc.scalar.activation(
                        slab[:, 2048 * half:2048 * half + 2048], pm_[:],
                        TANH, bias=0.0, scale=s_ss[:, 0:1])
                nc.vector.tensor_scalar_max(slab[:], slab[:], 0.0)
                nc.sync.dma_start(out=out_a.ap()[128 * i:128 * i + 128, :],
                                  in_=slab[:])

    nc.finalize()
    return nc


def _in_maps(spatial_nodes, temporal_nodes, ss1_w, ss2_w, st_w, st_b, ts_w, ts_b):
    f = np.float32
    maps = []
    wpack = np.zeros((128, WP), dtype=f)
    wpack[0:D, 0:D] = ss1_w.T
    wpack[0:D, D:2 * D] = ss2_w.T
    wpack[0:D, 64:64 + D] = ss2_w.T
    wpack[0:D, 64 + D:128] = ss1_w.T
    wpack[0:D, 128:130] = np.stack([st_w[0, D:], st_w[0, D:]], 1)
    wpack[0:D, 130:132] = np.stack([ts_w[0, D:], ts_w[0, D:]], 1)
    wpack[0:D, 132:134] = np.stack([st_w[0, :D], st_w[0, :D]], 1)
    wpack[0:D, 134:136] = np.stack([ts_w[0, :D], ts_w[0, :D]], 1)
    wpack[0:D, 137] = ALPHA
    wpack[D:2 * D, 137] = -ALPHA
    wpack[0, 138] = np.float32(np.asarray(st_b).reshape(-1)[0])
    wpack[0, 139] = np.float32(np.asarray(ts_b).reshape(-1)[0])
    for c in range(N_CORES):
        b, h = c // 2, c % 2
        wp = wpack.copy()
        wp[:, 136] = TB * h
        xs_b = np.asarray(spatial_nodes[b], dtype=f)
        xt_b = np.asarray(temporal_nodes[b], dtype=f)
        xs_rows = np.concatenate(
            [xs_b[128 * g:128 * g + 128] for g in GL[h]], 0)
        maps.append({
            "xsT": np.ascontiguousarray(xs_b.T),
            "xsrT": np.ascontiguousarray(xs_rows.T),
            "xtT": np.ascontiguousarray(xt_b.T),
            "xtrT": np.ascontiguousarray(xt_b[TB * h:TB * h + TB].T),
            "wpack": wp,
        })
    return maps


def run_kernel(inputs, trace=False, **spmd_kwargs):
    nc = _build_nc()
    maps = _in_maps(**inputs)
    res = bass_utils.run_bass_kernel_spmd(
        nc, maps, core_ids=list(range(N_CORES)), trace=trace, **spmd_kwargs)
    adj = np.empty((B, NT, NT), dtype=np.float32)
    for c in range(N_CORES):
        b, h = c // 2, c % 2
        oa = np.asarray(res.results[c]["out_a"], dtype=np.float32)
        ob = np.asarray(res.results[c]["out_b"], dtype=np.float32)
        for li, g in enumerate(GL[h]):
            adj[b, 128 * g:128 * g + 128, :] = oa[128 * li:128 * li + 128]
        adj[b, N + TB * h:N + TB * h + TB, :] = ob
    return adj, res


def kernel(**inputs):
    adj, _ = run_kernel(inputs, trace=False)
    return adj
